# revision 1
# baseline (speedup 1.0000x reference)
"""Trainium2 Bass kernel for nn_BasisDense: y = einsum('bd,duk,bk->bu', x, kernel, c_prob) + bias.

Strategy:
  - Factorize: t[b,(u,k)] = x @ kernel2d  (kernel2d = kernel.reshape(D, U*K), its
    NATURAL memory layout -> fully contiguous DMA of the kernel tensor), then
    y[b,u] = sum_k t[b,u,k]*c_prob[b,k] + bias[u] (cheap DVE epilogue).
  - Hybrid shard across 8 cores: batch B into 4 x units U into 2 (halves the
    per-core kernel-tensor HBM traffic vs pure batch sharding, keeping the two
    cores of each HBM pair well under the shared 716 GB/s).
  - Matmuls run in float32r (full PE speed; ~1.5e-4 rms rel err vs fp32).
  - Host-side input marshaling: x transposed to [D, BS] (lhsT layout), bias
    broadcast over the 128 partitions. O(B*D + U) work, negligible vs the
    O(B*D*U*K) kernel.
"""
import sys

sys.path.insert(0, "/opt/trn_rl_repo")

import numpy as np
import concourse.bacc as bacc
import concourse.mybir as mybir
import concourse.tile as tile
from concourse import bass_utils

B, D, U, K = 4096, 2048, 2048, 8
NCORES = 8
SHARD_U = 2  # units-dimension shards (1 = pure batch sharding)
SHARD_B = NCORES // SHARD_U
BS = B // SHARD_B  # batch rows per core
USH = U // SHARD_U  # units per core
UKS = USH * K  # fused (u,k) output columns per core
NFREE = 512  # matmul moving free dim (fp32 max, 1 PSUM bank)
NT = UKS // NFREE  # n-tiles
DT = D // 128  # contraction tiles
BT = BS // 128  # batch partition-tiles per core
UPT = NFREE // K  # u-columns produced per n-tile
KT_BUFS = 3

_CACHE = {}


def _build():
    nc = bacc.Bacc("TRN2", target_bir_lowering=False, debug=False, num_devices=NCORES)
    f32 = mybir.dt.float32
    f32r = mybir.dt.float32r

    xt = nc.dram_tensor("xt", [D, BS], f32r, kind="ExternalInput").ap()
    cp = nc.dram_tensor("cp", [BS, K], f32, kind="ExternalInput").ap()
    kern = nc.dram_tensor("kern", [D, USH, K], f32r, kind="ExternalInput").ap()
    biasr = nc.dram_tensor("biasr", [128, USH], f32, kind="ExternalInput").ap()
    y = nc.dram_tensor("y", [BS, USH], f32, kind="ExternalOutput").ap()

    # [128 d-partition, DT, UKS] view of this core's kernel2d shard
    kern2d = kern.rearrange("(t p) u k -> p t (u k)", p=128)

    with tile.TileContext(nc) as tc:
        with (
            tc.tile_pool(name="const", bufs=1) as constp,
            tc.tile_pool(name="kt", bufs=KT_BUFS) as ktp,
            tc.tile_pool(name="mps", bufs=8, space="PSUM") as mps,
            tc.tile_pool(name="ep", bufs=4) as epp,
            tc.tile_pool(name="yp", bufs=16) as ypp,
        ):
            xT = constp.tile([128, DT, BS], f32r)  # [d-part, d-tile, b]
            c_rep = constp.tile([128, BT, NFREE], f32)
            bias_rep = constp.tile([128, USH], f32)

            # xT rides the gpsimd (SWDGE) queue, off the two HWDGE queues
            # that carry the kernel-chunk stream
            xt_v = xt.rearrange("(t p) b -> p t b", p=128)
            c_nat = constp.tile([128, BT, K], f32)
            nc.scalar.dma_start(c_nat, cp.rearrange("(bt p) k -> p bt k", p=128))
            for t in range(DT):
                nc.gpsimd.dma_start(xT[:, t, :], xt_v[:, t, :])
            # replicate c_prob 64x along the free dim on the DVE (tiny)
            for bt in range(BT):
                nc.vector.tensor_copy(c_rep[:, bt, 0:K], c_nat[:, bt, :])
                s = K
                while s < NFREE:
                    nc.vector.tensor_copy(c_rep[:, bt, s : 2 * s], c_rep[:, bt, 0:s])
                    s *= 2

            for n in range(NT):
                kt = ktp.tile([128, DT, NFREE], f32r, tag="kt")
                # per-d-tile chunk DMAs (256KB each): the t-th matmul can
                # start as soon as chunk t lands, alternating across queues
                for t in range(DT):
                    eng = nc.sync if t % 2 == 0 else nc.scalar
                    eng.dma_start(
                        kt[:, t, :],
                        kern2d[:, t, n * NFREE : (n + 1) * NFREE],
                    )
                if n == 0:
                    # queued behind n=0's kernel chunks (frees the scalar
                    # queue's critical window) but emitted before any reader
                    nc.scalar.dma_start(bias_rep, biasr)
                for bt in range(BT):
                    acc = mps.tile([128, NFREE], f32, tag="acc")
                    for t in range(DT):
                        nc.tensor.matmul(
                            acc,
                            xT[:, t, bt * 128 : (bt + 1) * 128],
                            kt[:, t, :],
                            start=(t == 0),
                            stop=(t == DT - 1),
                        )
                    # epilogue: y[b, u] = sum_k acc[b, (u,k)] * c[b, k] + bias[u]
                    tmp = epp.tile([128, NFREE], f32, tag="tmp")
                    nc.vector.tensor_mul(tmp, acc, c_rep[:, bt, :])
                    yt = ypp.tile([128, UPT], f32, tag="yt")
                    nc.vector.tensor_reduce(
                        yt,
                        tmp.rearrange("p (u k) -> p u k", k=K),
                        axis=mybir.AxisListType.X,
                        op=mybir.AluOpType.add,
                    )
                    yf = ypp.tile([128, UPT], f32, tag="yf")
                    nc.vector.tensor_add(yf, yt, bias_rep[:, n * UPT : (n + 1) * UPT])
                    # output DMAs ride the scalar engine's HWDGE queue
                    nc.scalar.dma_start(
                        y[bt * 128 : (bt + 1) * 128, n * UPT : (n + 1) * UPT],
                        yf,
                    )
    nc.compile()
    return nc


def _in_maps(x, c_prob, kernel, bias):
    x = np.ascontiguousarray(x, dtype=np.float32)
    c_prob = np.ascontiguousarray(c_prob, dtype=np.float32)
    kernel = np.ascontiguousarray(kernel, dtype=np.float32)
    bias = np.ascontiguousarray(bias, dtype=np.float32)
    maps = []
    for c in range(NCORES):
        bq, uh = c % SHARD_B, c // SHARD_B
        xs = x[bq * BS : (bq + 1) * BS]
        maps.append(
            {
                "xt": np.ascontiguousarray(xs.T),
                "cp": c_prob[bq * BS : (bq + 1) * BS],
                "kern": np.ascontiguousarray(kernel[:, uh * USH : (uh + 1) * USH, :]),
                "biasr": np.ascontiguousarray(
                    np.broadcast_to(bias[uh * USH : (uh + 1) * USH], (128, USH))
                ),
            }
        )
    return maps


def kernel(x, c_prob, kernel, bias):
    if "nc" not in _CACHE:
        _CACHE["nc"] = _build()
    nc = _CACHE["nc"]
    res = bass_utils.run_bass_kernel_spmd(
        nc, _in_maps(x, c_prob, kernel, bias), list(range(NCORES))
    )
    out = np.empty((B, U), dtype=np.float32)
    for c in range(NCORES):
        bq, uh = c % SHARD_B, c // SHARD_B
        out[bq * BS : (bq + 1) * BS, uh * USH : (uh + 1) * USH] = res.results[c]["y"]
    return out



# revision 3
# speedup vs baseline: 1.1407x; 1.1407x over previous
"""Trainium2 Bass kernel for nn_BasisDense: y = einsum('bd,duk,bk->bu', x, kernel, c_prob) + bias.

Strategy (v2):
  - Factorize: t[b,(u,k)] = x @ kernel2d (kernel2d = kernel.reshape(D, U*K)),
    then y[b,u] = sum_k t[b,u,k]*c_prob[b,k] (DVE epilogue); bias added on host.
  - Mixed precision: first M d-tiles (of 16) of the contraction run as fp8e4
    DoubleRow matmuls (2 d-tiles per instruction, 2x PE rate); the rest in
    bf16 (full PE rate, ~half the f32r issue overhead and HBM bytes).
    All operands pre-scaled by powers of two (x*16, kernel*256 -- exact in
    bf16) so fp8 and bf16 products share one PSUM scale; the 1/4096 unscale
    is folded into c_prob on the host. M per n-tile via M_LIST.
  - Hybrid shard across 8 cores: batch B into 4 x units U into 2.
  - x rides the two HWDGE queues (sync/scalar) interleaved with the kernel
    chunk stream in consumption order (bt0's slices first).
  - ~18 warm-up matmuls on a zeroed SBUF tile run during the DMA head so the
    PE_HAM clock gate is released (2.4 GHz) before the real stream starts.
"""
import sys

sys.path.insert(0, "/opt/trn_rl_repo")

import numpy as np
import concourse.bacc as bacc
import concourse.mybir as mybir
import concourse.tile as tile
from concourse import bass_utils

B, D, U, K = 4096, 2048, 2048, 8
NCORES = 8
SHARD_U = 2  # units-dimension shards
SHARD_B = NCORES // SHARD_U
BS = B // SHARD_B  # batch rows per core
USH = U // SHARD_U  # units per core
UKS = USH * K  # fused (u,k) output columns per core
NFREE = 512  # matmul moving free dim (1 PSUM bank of fp32)
NT = UKS // NFREE  # n-tiles
DT = D // 128  # contraction d-tiles
BT = BS // 128  # batch partition-tiles per core
UPT = NFREE // K  # u-columns produced per n-tile
KT_BUFS = 4
WARM_MMS = 18  # PE warm-up matmuls issued during the DMA head

# fp8 d-tiles per n-tile (0, 2 or 4; DoubleRow pairs). Tune for rel-err vs
# speed: m=2 for all -> ~1.34e-2, m=4 for all -> ~1.88e-2 (gate 2e-2).
M_LIST = [2] * NT
MF8 = 4  # d-tiles shipped in fp8 (max of M_LIST coverage)
D8 = MF8 * 128  # fp8 d-rows
D16_OFF = 2 * 128  # bf16 rows start at d-tile 2 (tiles 2..15 always in bf16)
DT16 = DT - 2

XSCALE = 16.0  # x pre-scale (power of 2: exact in bf16)
WSCALE = 256.0  # kernel pre-scale
F8 = mybir.dt.float8e4
BF16 = mybir.dt.bfloat16

_CACHE = {}


def _build():
    nc = bacc.Bacc("TRN2", target_bir_lowering=False, debug=False, num_devices=NCORES)
    f32 = mybir.dt.float32
    DR = mybir.MatmulPerfMode.DoubleRow

    xt8 = nc.dram_tensor("xt8", [D8, BS], F8, kind="ExternalInput").ap()
    xt16 = nc.dram_tensor("xt16", [DT16 * 128, BS], BF16, kind="ExternalInput").ap()
    cp = nc.dram_tensor("cp", [BS, K], f32, kind="ExternalInput").ap()
    kern8 = nc.dram_tensor("kern8", [D8, USH, K], F8, kind="ExternalInput").ap()
    kern16 = nc.dram_tensor("kern16", [DT16 * 128, USH, K], BF16, kind="ExternalInput").ap()
    y = nc.dram_tensor("y", [BS, USH], f32, kind="ExternalOutput").ap()

    x8v = xt8.rearrange("(t p) b -> p t b", p=128)  # [128, 4, BS]
    x16v = xt16.rearrange("(t p) b -> p t b", p=128)  # [128, 14, BS]
    k8v = kern8.rearrange("(t p) u k -> p t (u k)", p=128)  # [128, 4, UKS]
    k16v = kern16.rearrange("(t p) u k -> p t (u k)", p=128)  # [128, 14, UKS]

    with tile.TileContext(nc) as tc:
        with (
            tc.tile_pool(name="const", bufs=1) as constp,
            tc.tile_pool(name="kt8p", bufs=KT_BUFS) as ktp8,
            tc.tile_pool(name="kt16p", bufs=KT_BUFS) as ktp16,
            tc.tile_pool(name="warmps", bufs=1, space="PSUM") as wps,
            tc.tile_pool(name="mps", bufs=7, space="PSUM") as mps,
            tc.tile_pool(name="ep", bufs=4) as epp,
            tc.tile_pool(name="yp", bufs=16) as ypp,
        ):
            xT8 = constp.tile([128, MF8, BS], F8)
            xT16 = constp.tile([128, DT16, BS], BF16)
            c_nat = constp.tile([128, BT, K], f32)
            c_rep = constp.tile([128, BT, NFREE], f32)
            warm = constp.tile([128, NFREE], BF16)

            # c_prob first on the scalar queue (tiny, needed by bt0 epilogue)
            nc.scalar.dma_start(c_nat, cp.rearrange("(bt p) k -> p bt k", p=128))

            # PE warm-up: release the HAM clock gate during the DMA head
            nc.vector.memset(warm, 0.0)
            wpsum = wps.tile([128, NFREE], f32)
            for _ in range(WARM_MMS):
                nc.tensor.matmul(wpsum, warm[:, 0:128], warm, start=True, stop=True)

            # replicate c_prob 64x along the free dim on the DVE (tiny)
            for bt in range(BT):
                nc.vector.tensor_copy(c_rep[:, bt, 0:K], c_nat[:, bt, :])
                s = K
                while s < NFREE:
                    nc.vector.tensor_copy(c_rep[:, bt, s : 2 * s], c_rep[:, bt, 0:s])
                    s *= 2

            def fetch(n, first=False):
                m = M_LIST[n]
                kt8 = ktp8.tile([128, MF8, NFREE], F8, tag="kt8")
                kt16 = ktp16.tile([128, DT16, NFREE], BF16, tag="kt16")
                nsl = slice(n * NFREE, (n + 1) * NFREE)
                nc.sync.dma_start(kt8[:, 0:m, :], k8v[:, 0:m, nsl])
                if first:
                    # bt0's x slices ride along, interleaved chunk-by-chunk
                    nc.scalar.dma_start(xT8[:, :, 0:128], x8v[:, :, 0:128])
                for t in range(m, DT):
                    eng = nc.sync if t % 2 == 0 else nc.scalar
                    eng.dma_start(kt16[:, t - m, :], k16v[:, t - 2, nsl])
                    if first:
                        oth = nc.scalar if t % 2 == 0 else nc.sync
                        oth.dma_start(xT16[:, t - 2, 0:128], x16v[:, t - 2, 0:128])
                return kt8, kt16

            kt_first = fetch(0, first=True)

            # remaining x slices (bt 1..7), alternating queues
            for bt in range(1, BT):
                bsl = slice(bt * 128, (bt + 1) * 128)
                e1 = nc.sync if bt % 2 == 0 else nc.scalar
                e2 = nc.scalar if bt % 2 == 0 else nc.sync
                e1.dma_start(xT8[:, :, bsl], x8v[:, :, bsl])
                e2.dma_start(xT16[:, :, bsl], x16v[:, :, bsl])

            for n in range(NT):
                m = M_LIST[n]
                kt8, kt16 = kt_first if n == 0 else fetch(n)
                for bt in range(BT):
                    bsl = slice(bt * 128, (bt + 1) * 128)
                    acc = mps.tile([128, NFREE], f32, tag="acc")
                    for j in range(m // 2):
                        nc.tensor.matmul(
                            acc,
                            xT8[:, 2 * j : 2 * j + 2, bsl],
                            kt8[:, 2 * j : 2 * j + 2, :],
                            start=(j == 0),
                            stop=False,
                            perf_mode=DR,
                        )
                    for t in range(m, DT):
                        nc.tensor.matmul(
                            acc,
                            xT16[:, t - 2, bsl],
                            kt16[:, t - m, :],
                            start=False,
                            stop=(t == DT - 1),
                        )
                    # epilogue: y[b, u] = sum_k acc[b, (u,k)] * c[b, k]
                    tmp = epp.tile([128, NFREE], f32, tag="tmp")
                    nc.vector.tensor_mul(tmp, acc, c_rep[:, bt, :])
                    yt = ypp.tile([128, UPT], f32, tag="yt")
                    nc.vector.tensor_reduce(
                        yt,
                        tmp.rearrange("p (u k) -> p u k", k=K),
                        axis=mybir.AxisListType.X,
                        op=mybir.AluOpType.add,
                    )
                    nc.scalar.dma_start(
                        y[bsl, n * UPT : (n + 1) * UPT],
                        yt,
                    )
    nc.compile()
    return nc


def _in_maps(x, c_prob, kernel, bias):
    np8 = mybir.dt.np(F8)
    np16 = mybir.dt.np(BF16)
    x = np.asarray(x, dtype=np.float32)
    c_prob = np.asarray(c_prob, dtype=np.float32)
    kernel = np.asarray(kernel, dtype=np.float32)

    xs = (x.T * np.float32(XSCALE))  # [D, B]
    x8_full = xs[:D8].astype(np8)
    x16_full = xs[D16_OFF:].astype(np16)
    ks = kernel * np.float32(WSCALE)
    k8_full = ks[:D8].astype(np8)  # [D8, U, K]
    k16_full = ks[D16_OFF:].astype(np16)
    cps = c_prob * np.float32(1.0 / (XSCALE * WSCALE))

    maps = []
    for c in range(NCORES):
        bq, uh = c % SHARD_B, c // SHARD_B
        bsl = slice(bq * BS, (bq + 1) * BS)
        usl = slice(uh * USH, (uh + 1) * USH)
        maps.append(
            {
                "xt8": np.ascontiguousarray(x8_full[:, bsl]),
                "xt16": np.ascontiguousarray(x16_full[:, bsl]),
                "cp": np.ascontiguousarray(cps[bsl]),
                "kern8": np.ascontiguousarray(k8_full[:, usl, :]),
                "kern16": np.ascontiguousarray(k16_full[:, usl, :]),
            }
        )
    return maps


def _gather(results, bias):
    out = np.empty((B, U), dtype=np.float32)
    for c in range(NCORES):
        bq, uh = c % SHARD_B, c // SHARD_B
        out[bq * BS : (bq + 1) * BS, uh * USH : (uh + 1) * USH] = results[c]["y"]
    out += np.asarray(bias, dtype=np.float32)
    return out


def kernel(x, c_prob, kernel, bias):
    if "nc" not in _CACHE:
        _CACHE["nc"] = _build()
    nc = _CACHE["nc"]
    res = bass_utils.run_bass_kernel_spmd(
        nc, _in_maps(x, c_prob, kernel, bias), list(range(NCORES))
    )
    return _gather(res.results, bias)
